# revision 34
# baseline (speedup 1.0000x reference)
"""Trainium2 Bass kernel for nn_CeptaContextBlock (B=4, T=4096, D=1024, P=512, ALPHA=4, PR=64).

Math (after algebraic simplification of the reference):
    W_comb = W_toP + sum_a W_U[:,:,a] * W_V[:,a]          (host precompute)
    WB     = W_comb @ B_mat                               (host precompute)
    Fg   = sigmoid(x @ W_F)                               (B,T,P)
    lam  = sigmoid(Fg @ W_lam)                            (B,T,PR)
    u    = x @ WB          (== (x @ W_comb) @ B_mat)      (B,T,PR)
    s    = scan: s_i = lam_i * s_{i-1} + u_i along T      (B,T,PR)
    t_tilde = x @ W_comb + s @ C_mat                      (B,T,P)
    h    = t_tilde @ W_fromP                              (B,T,D)

Sharding: 8 cores; core c handles batch b=c//2, token half c%2 (2048 tokens).
No collective: each core redundantly recomputes the scan over a WU=64-token
warmup window that precedes its token range (zeros for the first half, so the
warmup scan stays exactly 0 there). The scan's decay products cumprod(lam)
shrink ~2^-1 per token (lam = sigmoid of an O(0.6)-std Gaussian; worst case
over 64 tokens still ~2^-50), so the warmup state converges to the true carry
far below fp32 noise. Everything is forward-local; no cross-core traffic.

The Fg matmul runs in fp8 e4m3 DoubleRow mode (two 128-deep k-tiles per
instruction -> 2x PE rate). Fg only steers the decay lam through two sigmoids,
so the ~3% fp8 quantization noise is attenuated to ~0.5% on the s-path and
~0.4% on h — well inside the 2e-2 gate. The main t path stays bf16.

Phases (per core, extended tokens TE = 64 warmup + 2048 main):
  A: per 512-token chunk: Fg matmuls (fp8) -> sigmoid -> lam matmuls ->
     sigmoid, u matmuls (bf16), chained tensor_tensor_scan (u from PSUM).
  B: t_tilde accumulated fully in PSUM: 8 k-chunk matmuls of x @ W_comb plus
     one accumulating matmul of C^T into the same bank; single cast to SBUF.
  H: h tiles (128 tokens) = ttil^T @ W_fromP into 2-bank PSUM tiles, one
     [128,1024] Vector copy each (Scalar's hwdge queue stays free so output
     piece DMA waits cannot stall the copy stream); output DMAed in 6
     batched pieces alternating sync/scalar, per-piece SBUF tiles keeping
     the DMA dependencies exact, the last piece a single tile.

DMA queues: scalar carries the fp8 weights + fp8 x stream; sync carries the
bf16 x stream, the deferred phase-B/H weights and its share of the output
pieces; gpsimd (software DGE, ~single-ring bandwidth) only the tiny wb/wlam.
"""

import os
import sys

import numpy as np

for _p in ("/opt/trn_rl_repo", "/root/.axon_site/_ro/trn_rl_repo"):
    if os.path.isdir(_p) and _p not in sys.path:
        sys.path.append(_p)

import ml_dtypes

import concourse.bass as bass
import concourse.bacc as bacc
import concourse.mybir as mybir
import concourse.tile as tile
from concourse import bass_utils

B, T, D, P, ALPHA, PR = 4, 4096, 1024, 512, 4, 64
NCORES = 8
TL = T // 2          # main tokens per core
WU = 64              # warmup tokens (scan converges to the true carry:
                     # worst-case decay over 64 steps is still ~2^-50)
TE = TL + WU         # extended tokens processed by phase A
KD = D // 128        # 8 d-chunks (contraction for the big matmuls)
KQ = KD // 2         # 4 fp8 DoubleRow k-pair tiles
PT = P // 128        # 4 p-tiles
CH = 512             # token chunk (free dim per matmul, fills one PSUM bank)
CHW = [512, 512, 512, 512, 64]             # phase-A chunk widths (sum = TE)
COFF = [0, 512, 1024, 1536, 2048]          # ext-frame chunk offsets
# phase-B groups: (chunk, token offset inside chunk, main-frame offset, width)
BGRP = [(0, WU, 0, 512 - WU), (1, 0, 448, 512), (2, 0, 960, 512),
        (3, 0, 1472, 512), (4, 0, 1984, 64)]
HT = TL // 128       # 16 h tiles
HPIECES = [(0, 4), (4, 8), (8, 12), (12, 14), (14, 15), (15, 16)]
F32 = mybir.dt.float32
BF16 = mybir.dt.bfloat16
F8 = mybir.dt.float8e4
SIG = mybir.ActivationFunctionType.Sigmoid
CPY = mybir.ActivationFunctionType.Copy
MUL = mybir.AluOpType.mult
ADD = mybir.AluOpType.add
DR = mybir.MatmulPerfMode.DoubleRow

_CACHE = {}


def _pairs(ap2d, w):
    """View a [128, 2*w] slice as [128, 2, w] for DoubleRow operands."""
    return ap2d.rearrange("p (i t) -> p i t", i=2)


def build_program(ncores: int = NCORES):
    """Build the SPMD Tile program (same NEFF on all cores)."""
    nc = bacc.Bacc(
        "TRN2", target_bir_lowering=False, debug=False, num_devices=ncores
    )

    # big inputs are pre-swizzled on the host to partition-major layout so
    # every DMA lands as 128 fully-contiguous per-partition runs
    xt_d = nc.dram_tensor("xt", [128, KD * TE], BF16, kind="ExternalInput")
    xf8_d = nc.dram_tensor("xf8", [128, KD * TE], F8, kind="ExternalInput")
    # W_F fp8, m-major DoubleRow pairs: [(m*KQ+q)*256 + i*128 + j]
    wf8_d = nc.dram_tensor("wf8", [128, PT * KD * 128], F8, kind="ExternalInput")
    wb_d = nc.dram_tensor("wb", [128, KD * PR], BF16, kind="ExternalInput")
    wcomb_d = nc.dram_tensor("wcomb", [128, KD * P], BF16, kind="ExternalInput")
    # W_lam fp8 DoubleRow pairs over the P contraction: [q*256 + i*128 + r]
    wlam_d = nc.dram_tensor("wlam", [128, PT * PR], F8, kind="ExternalInput")
    cmat_d = nc.dram_tensor("cmat", [PR, P], BF16, kind="ExternalInput")
    wfp_d = nc.dram_tensor("wfp", [P, D], BF16, kind="ExternalInput")
    h_d = nc.dram_tensor("h", [TL, D], BF16, kind="ExternalOutput")

    wfp_v = wfp_d.rearrange("(k p) q -> p k q", p=128)     # [128, PT, D]
    h_v = h_d.rearrange("(n p) d -> p n d", p=128)         # [128, HT, D]

    with tile.TileContext(nc) as tc:
        with (
            tc.tile_pool(name="wp", bufs=1) as wp,
            tc.tile_pool(name="xp", bufs=1) as xp,
            tc.tile_pool(name="big", bufs=1) as big,
            tc.tile_pool(name="ppa", bufs=2, space="PSUM") as ppa,
            tc.tile_pool(name="pps", bufs=2, space="PSUM") as pps,
            tc.tile_pool(name="pph", bufs=2, space="PSUM") as pph,
        ):
            # ---- input loads. The scalar hwdge queue starts fastest and
            # carries the Fg/scan-critical stream (fp8 weights + fp8 x + the
            # tiny u/lam weights), all issued before phase A's first sigmoid
            # can occupy the queue. The sync hwdge queue (DMA-only, ~7us
            # startup) carries the bf16 x stream and the deferred phase-B/H
            # weights. gpsimd software-DGE is ~single-ring slow — unused.
            wf8_sb = wp.tile([128, PT * KD * 128], F8, tag="wf8", name="wf8_sb")
            wb_sb = wp.tile([128, KD * PR], BF16, tag="wb", name="wb_sb")
            wlam_sb = wp.tile([128, PT * PR], F8, tag="wlam", name="wlam_sb")
            warm_sb = wp.tile([128, 128], BF16, tag="warm", name="warm_sb")
            wout_sb = wp.tile([128, 128], BF16, tag="wout", name="wout_sb")
            xf8_tiles, xt_tiles = [], []
            for c in range(5):
                w = KD * CHW[c]
                xf8_c = xp.tile([128, w], F8, tag=f"xf8_{c}", name=f"xf8_{c}")
                xt_c = xp.tile([128, w], BF16, tag=f"xt{c}", name=f"xt{c}")
                xf8_tiles.append(xf8_c)
                xt_tiles.append(xt_c)
            nc.scalar.dma_start(wf8_sb[:, :2048], wf8_d[:, :2048])
            nc.sync.dma_start(xf8_tiles[0][:], xf8_d[:, : KD * CHW[0]])
            nc.scalar.dma_start(wf8_sb[:, 2048:], wf8_d[:, 2048:])
            nc.gpsimd.dma_start(wb_sb[:], wb_d[:, :])
            nc.gpsimd.dma_start(wlam_sb[:], wlam_d[:, :])
            for c in range(1, 5):
                o, w = KD * COFF[c], KD * CHW[c]
                nc.scalar.dma_start(xf8_tiles[c][:], xf8_d[:, o : o + w])
            wcomb_sb = wp.tile([128, KD * P], BF16, tag="wcomb", name="wcomb_sb")
            cmat_sb = wp.tile([PR, P], BF16, tag="cmat", name="cmat_sb")
            wfp_sb = wp.tile([128, PT * D], BF16, tag="wfp", name="wfp_sb")
            for c in range(5):
                o, w = KD * COFF[c], KD * CHW[c]
                nc.sync.dma_start(xt_tiles[c][:], xt_d[:, o : o + w])
                if c == 1:
                    nc.sync.dma_start(wcomb_sb[:], wcomb_d[:, :])
            nc.sync.dma_start(cmat_sb[:], cmat_d[:, :])
            nc.sync.dma_start(
                wfp_sb[:].rearrange("p (k q) -> p k q", k=PT), wfp_v[:, :, :]
            )

            # warm the PE pipeline and the sigmoid table while DMAs land
            nc.vector.memset(warm_sb[:], 0.0)
            pw = ppa.tile([128, CH], F32, tag="pa", name="pw")
            for i in range(8):
                nc.tensor.matmul(
                    pw[:, :128], warm_sb[:], warm_sb[:],
                    start=(i == 0), stop=(i == 7),
                )
            nc.scalar.activation(wout_sb[:, 0:1], warm_sb[:, 0:1], SIG)

            # ---- persistent activations ----
            # Fg stored as fp8 DoubleRow pairs (only consumer is the lam mm)
            fgp_sb = [
                big.tile([128, 2, TE], F8, tag=f"fgp{q2}", name=f"fgp{q2}")
                for q2 in range(2)
            ]
            ttil_sb = [
                big.tile([128, TL], BF16, tag=f"ttil{m}", name=f"ttil{m}")
                for m in range(PT)
            ]
            lam_sb = big.tile([PR, TE], F32, tag="lam", name="lam")
            s1_sb = big.tile([PR, TE], F32, tag="s1", name="s1")
            sloc_sb = big.tile([PR, TL], BF16, tag="sloc", name="sloc")
            h_sb = [
                big.tile([128, hi - lo, D], BF16, tag=f"hs{i}", name=f"hs{i}")
                for i, (lo, hi) in enumerate(HPIECES)
            ]

            # ---- phase A: Fg (fp8), lam, u, chained scan over ext tokens ----
            for c in range(5):
                co, w = COFF[c], CHW[c]
                xf8_c, xt_c = xf8_tiles[c], xt_tiles[c]
                for m in range(PT):
                    pa = ppa.tile([128, CH], F32, tag="pa", name=f"pa{c}_{m}")
                    for q in range(KQ):
                        nc.tensor.matmul(
                            pa[:, :w],
                            _pairs(
                                wf8_sb[:, (m * KQ + q) * 256 : (m * KQ + q + 1) * 256],
                                128,
                            ),
                            _pairs(xf8_c[:, q * 2 * w : (q + 1) * 2 * w], w),
                            start=(q == 0),
                            stop=(q == KQ - 1),
                            perf_mode=DR,
                        )
                    nc.scalar.activation(
                        fgp_sb[m // 2][:, m % 2, co : co + w], pa[:, :w], SIG
                    )
                # u = x @ WB (bf16, 64 wide)
                pu = pps.tile([PR, CH], F32, tag="ps", name=f"pu{c}")
                for k in range(KD):
                    nc.tensor.matmul(
                        pu[:, :w],
                        wb_sb[:, k * PR : (k + 1) * PR],
                        xt_c[:, k * w : (k + 1) * w],
                        start=(k == 0),
                        stop=(k == KD - 1),
                    )
                # lam = sigmoid(Fg @ W_lam), fp8 DoubleRow over the P pairs
                pl = pps.tile([PR, CH], F32, tag="ps", name=f"pl{c}")
                for q2 in range(2):
                    nc.tensor.matmul(
                        pl[:, :w],
                        _pairs(wlam_sb[:, q2 * 128 : (q2 + 1) * 128], PR),
                        fgp_sb[q2][:, :, co : co + w],
                        start=(q2 == 0),
                        stop=(q2 == 1),
                        perf_mode=DR,
                    )
                nc.scalar.activation(lam_sb[:, co : co + w], pl[:, :w], SIG)
                # chained scan; u consumed straight from PSUM
                init = 0.0 if c == 0 else s1_sb[:, co - 1 : co]
                nc.vector.tensor_tensor_scan(
                    s1_sb[:, co : co + w], lam_sb[:, co : co + w], pu[:, :w],
                    init, op0=MUL, op1=ADD,
                )
                # main-frame bf16 copy of the scan state for the C_mat matmul
                if c == 0:
                    nc.vector.tensor_copy(sloc_sb[:, 0 : w - WU], s1_sb[:, WU:w])
                else:
                    nc.vector.tensor_copy(
                        sloc_sb[:, co - WU : co - WU + w], s1_sb[:, co : co + w]
                    )

            # ---- phase B: t_tilde = x @ W_comb + s @ C, all in PSUM ----
            for g, (c, xo, mo, w) in enumerate(BGRP):
                xt_c = xt_tiles[c]
                cw = CHW[c]
                for m in range(PT):
                    pb = ppa.tile([128, CH], F32, tag="pa", name=f"pb{g}_{m}")
                    for k in range(KD):
                        nc.tensor.matmul(
                            pb[:, :w],
                            wcomb_sb[:, k * P + m * 128 : k * P + (m + 1) * 128],
                            xt_c[:, k * cw + xo : k * cw + xo + w],
                            start=(k == 0),
                            stop=False,
                        )
                    nc.tensor.matmul(
                        pb[:, :w],
                        cmat_sb[:, m * 128 : (m + 1) * 128],
                        sloc_sb[:, mo : mo + w],
                        start=False,
                        stop=True,
                    )
                    nc.vector.tensor_copy(ttil_sb[m][:, mo : mo + w], pb[:, :w])

            # ---- phase H: h tiles, batched output DMA. All PSUM->SBUF copies
            # run on Vector so the Scalar hwdge queue stays free for its share
            # of the output piece DMAs (piece waits there would stall copies).
            for pi, (lo, hi) in enumerate(HPIECES):
                last = pi == len(HPIECES) - 1
                for tt in range(lo, hi):
                    ts_ = slice(tt * 128, (tt + 1) * 128)
                    ph = pph.tile([128, D], F32, tag="ph", name=f"ph{tt}")
                    for dc in range(2):
                        for k in range(PT):
                            nc.tensor.matmul(
                                ph[:, dc * CH : (dc + 1) * CH],
                                ttil_sb[k][:, ts_],
                                wfp_sb[:, k * D + dc * CH : k * D + dc * CH + CH],
                                start=(k == 0),
                                stop=(k == PT - 1),
                            )
                        if last:
                            # half-granular copies shorten the final
                            # matmul -> copy -> DMA chain by ~0.6us
                            nc.vector.tensor_copy(
                                h_sb[pi][:, tt - lo, dc * CH : (dc + 1) * CH],
                                ph[:, dc * CH : (dc + 1) * CH],
                            )
                    if not last:
                        nc.vector.tensor_copy(h_sb[pi][:, tt - lo, :], ph[:])
                eng = nc.sync if pi % 2 == 0 else nc.scalar
                eng.dma_start(h_v[:, lo:hi, :], h_sb[pi][:, :, :])

    nc.compile()
    return nc


def _prep_inputs(x, W_toP, W_U, W_F, W_V, W_lam, B_mat, C_mat, W_fromP):
    """Host-side sharding prep: weight folds, dtype casts, per-core x swizzle."""
    bf = ml_dtypes.bfloat16
    f8 = ml_dtypes.float8_e4m3fn

    def swz(w):
        # [K*128, q] -> partition-major [128, K*q]
        kq = w.shape[0] // 128
        return np.ascontiguousarray(
            w.reshape(kq, 128, w.shape[1]).transpose(1, 0, 2).reshape(128, -1)
        )

    W_comb = (W_toP + (W_U * W_V[None, :, :]).sum(-1)).astype(np.float32)
    WB = W_comb @ np.asarray(B_mat, np.float32)
    # W_F fp8 m-major DoubleRow pairs: wf8[p, ((m*KQ+q)*2+i)*128+j]
    wf = np.asarray(W_F, np.float32).reshape(KQ, 2, 128, PT, 128)
    wf8 = np.ascontiguousarray(
        wf.transpose(2, 3, 0, 1, 4).reshape(128, -1)
    ).astype(f8)
    wb = swz(WB).astype(bf)
    wcomb = swz(W_comb).astype(bf)
    # W_lam fp8 DoubleRow pairs: wlam8[p, (q*2+i)*64+r] = W_lam[(2q+i)*128+p, r]
    wl = np.asarray(W_lam, np.float32).reshape(2, 2, 128, PR)
    wlam = np.ascontiguousarray(
        wl.transpose(2, 0, 1, 3).reshape(128, -1)
    ).astype(f8)
    cmat = np.asarray(C_mat, np.float32).astype(bf)
    wfp = np.asarray(W_fromP, np.float32).astype(bf)
    in_maps = []
    for c in range(NCORES):
        b, half = c // 2, c % 2
        xm = np.asarray(x[b, half * TL : (half + 1) * TL, :], np.float32)
        if half == 0:
            warm = np.zeros((WU, D), np.float32)
        else:
            warm = np.asarray(x[b, TL - WU : TL, :], np.float32)
        xT = np.concatenate([warm, xm], axis=0).T       # [D, TE]
        pieces, pieces8 = [], []
        for ci in range(5):
            blk = xT[:, COFF[ci] : COFF[ci] + CHW[ci]]
            # bf16: [D, w] -> [128, KD*w] k-major partition-major
            pieces.append(
                blk.reshape(KD, 128, CHW[ci]).transpose(1, 0, 2).reshape(128, -1)
            )
            # fp8 DoubleRow: [128, (q*2+i)*w + t]
            pieces8.append(
                blk.reshape(KQ, 2, 128, CHW[ci]).transpose(2, 0, 1, 3).reshape(128, -1)
            )
        xs = np.ascontiguousarray(np.concatenate(pieces, axis=1)).astype(bf)
        xs8 = np.ascontiguousarray(np.concatenate(pieces8, axis=1)).astype(f8)
        in_maps.append(
            {
                "xt": xs,
                "xf8": xs8,
                "wf8": wf8,
                "wb": wb,
                "wcomb": wcomb,
                "wlam": wlam,
                "cmat": cmat,
                "wfp": wfp,
            }
        )
    return in_maps


def kernel(**inputs) -> np.ndarray:
    inputs = {k: np.asarray(v) for k, v in inputs.items()}
    if "nc" not in _CACHE:
        _CACHE["nc"] = build_program()
    nc = _CACHE["nc"]
    in_maps = _prep_inputs(**inputs)
    trace = bool(int(os.environ.get("CEPTA_TRACE", "0")))
    res = bass_utils.run_bass_kernel_spmd(
        nc,
        in_maps,
        core_ids=list(range(NCORES)),
        trace=trace,
        trace_cores=[0] if trace else None,
    )
    _CACHE["last_result"] = res
    out = np.empty((B, T, D), np.float32)
    for c in range(NCORES):
        b, half = c // 2, c % 2
        out[b, half * TL : (half + 1) * TL, :] = res.results[c]["h"].astype(
            np.float32
        )
    return out


# revision 35
# speedup vs baseline: 1.0038x; 1.0038x over previous
"""Trainium2 Bass kernel for nn_CeptaContextBlock (B=4, T=4096, D=1024, P=512, ALPHA=4, PR=64).

Math (after algebraic simplification of the reference):
    W_comb = W_toP + sum_a W_U[:,:,a] * W_V[:,a]          (host precompute)
    WB     = W_comb @ B_mat                               (host precompute)
    Fg   = sigmoid(x @ W_F)                               (B,T,P)
    lam  = sigmoid(Fg @ W_lam)                            (B,T,PR)
    u    = x @ WB          (== (x @ W_comb) @ B_mat)      (B,T,PR)
    s    = scan: s_i = lam_i * s_{i-1} + u_i along T      (B,T,PR)
    t_tilde = x @ W_comb + s @ C_mat                      (B,T,P)
    h    = t_tilde @ W_fromP                              (B,T,D)

Sharding: 8 cores; core c handles batch b=c//2, token half c%2 (2048 tokens).
No collective: each core redundantly recomputes the scan over a WU=64-token
warmup window that precedes its token range (zeros for the first half, so the
warmup scan stays exactly 0 there). The scan's decay products cumprod(lam)
shrink ~2^-1 per token (lam = sigmoid of an O(0.6)-std Gaussian; worst case
over 64 tokens still ~2^-50), so the warmup state converges to the true carry
far below fp32 noise. Everything is forward-local; no cross-core traffic.

The Fg matmul runs in fp8 e4m3 DoubleRow mode (two 128-deep k-tiles per
instruction -> 2x PE rate). Fg only steers the decay lam through two sigmoids,
so the ~3% fp8 quantization noise is attenuated to ~0.5% on the s-path and
~0.4% on h — well inside the 2e-2 gate. The main t path stays bf16.

Phases (per core, extended tokens TE = 64 warmup + 2048 main):
  A: per 512-token chunk: Fg matmuls (fp8) -> sigmoid -> lam matmuls ->
     sigmoid, u matmuls (bf16), chained tensor_tensor_scan (u from PSUM).
  B: t_tilde accumulated fully in PSUM: 8 k-chunk matmuls of x @ W_comb plus
     one accumulating matmul of C^T into the same bank; single cast to SBUF.
  H: h tiles (128 tokens) = ttil^T @ W_fromP into 2-bank PSUM tiles, one
     [128,1024] Vector copy each (Scalar's hwdge queue stays free so output
     piece DMA waits cannot stall the copy stream); output DMAed in 6
     batched pieces alternating sync/scalar, per-piece SBUF tiles keeping
     the DMA dependencies exact, the last piece a single tile.

DMA queues: scalar carries the fp8 weights + fp8 x stream; sync carries the
bf16 x stream, the deferred phase-B/H weights and its share of the output
pieces; gpsimd (software DGE, ~single-ring bandwidth) only the tiny wb/wlam.
"""

import os
import sys

import numpy as np

for _p in ("/opt/trn_rl_repo", "/root/.axon_site/_ro/trn_rl_repo"):
    if os.path.isdir(_p) and _p not in sys.path:
        sys.path.append(_p)

import ml_dtypes

import concourse.bass as bass
import concourse.bacc as bacc
import concourse.mybir as mybir
import concourse.tile as tile
from concourse import bass_utils

B, T, D, P, ALPHA, PR = 4, 4096, 1024, 512, 4, 64
NCORES = 8
TL = T // 2          # main tokens per core
WU = 64              # warmup tokens (scan converges to the true carry:
                     # worst-case decay over 64 steps is still ~2^-50)
TE = TL + WU         # extended tokens processed by phase A
KD = D // 128        # 8 d-chunks (contraction for the big matmuls)
KQ = KD // 2         # 4 fp8 DoubleRow k-pair tiles
PT = P // 128        # 4 p-tiles
CH = 512             # token chunk (free dim per matmul, fills one PSUM bank)
CHW = [512, 512, 512, 512, 64]             # phase-A chunk widths (sum = TE)
COFF = [0, 512, 1024, 1536, 2048]          # ext-frame chunk offsets
# phase-B groups: (chunk, token offset inside chunk, main-frame offset, width)
BGRP = [(0, WU, 0, 512 - WU), (1, 0, 448, 512), (2, 0, 960, 512),
        (3, 0, 1472, 512), (4, 0, 1984, 64)]
HT = TL // 128       # 16 h tiles
HPIECES = [(0, 4), (4, 8), (8, 12), (12, 14), (14, 15), (15, 16)]
F32 = mybir.dt.float32
BF16 = mybir.dt.bfloat16
F8 = mybir.dt.float8e4
SIG = mybir.ActivationFunctionType.Sigmoid
CPY = mybir.ActivationFunctionType.Copy
MUL = mybir.AluOpType.mult
ADD = mybir.AluOpType.add
DR = mybir.MatmulPerfMode.DoubleRow

_CACHE = {}


def _pairs(ap2d, w):
    """View a [128, 2*w] slice as [128, 2, w] for DoubleRow operands."""
    return ap2d.rearrange("p (i t) -> p i t", i=2)


def build_program(ncores: int = NCORES):
    """Build the SPMD Tile program (same NEFF on all cores)."""
    nc = bacc.Bacc(
        "TRN2", target_bir_lowering=False, debug=False, num_devices=ncores
    )

    # big inputs are pre-swizzled on the host to partition-major layout so
    # every DMA lands as 128 fully-contiguous per-partition runs
    xt_d = nc.dram_tensor("xt", [128, KD * TE], BF16, kind="ExternalInput")
    xf8_d = nc.dram_tensor("xf8", [128, KD * TE], F8, kind="ExternalInput")
    # W_F fp8, m-major DoubleRow pairs: [(m*KQ+q)*256 + i*128 + j]
    wf8_d = nc.dram_tensor("wf8", [128, PT * KD * 128], F8, kind="ExternalInput")
    wb_d = nc.dram_tensor("wb", [128, KD * PR], BF16, kind="ExternalInput")
    wcomb_d = nc.dram_tensor("wcomb", [128, KD * P], BF16, kind="ExternalInput")
    # W_lam fp8 DoubleRow pairs over the P contraction: [q*256 + i*128 + r]
    wlam_d = nc.dram_tensor("wlam", [128, PT * PR], F8, kind="ExternalInput")
    cmat_d = nc.dram_tensor("cmat", [PR, P], BF16, kind="ExternalInput")
    wfp_d = nc.dram_tensor("wfp", [P, D], BF16, kind="ExternalInput")
    h_d = nc.dram_tensor("h", [TL, D], BF16, kind="ExternalOutput")

    wfp_v = wfp_d.rearrange("(k p) q -> p k q", p=128)     # [128, PT, D]
    h_v = h_d.rearrange("(n p) d -> p n d", p=128)         # [128, HT, D]

    with tile.TileContext(nc) as tc:
        with (
            tc.tile_pool(name="wp", bufs=1) as wp,
            tc.tile_pool(name="xp", bufs=1) as xp,
            tc.tile_pool(name="big", bufs=1) as big,
            tc.tile_pool(name="ppa", bufs=2, space="PSUM") as ppa,
            tc.tile_pool(name="pps", bufs=2, space="PSUM") as pps,
            tc.tile_pool(name="pph", bufs=2, space="PSUM") as pph,
        ):
            # ---- input loads. The scalar hwdge queue starts fastest and
            # carries the Fg/scan-critical stream (fp8 weights + fp8 x + the
            # tiny u/lam weights), all issued before phase A's first sigmoid
            # can occupy the queue. The sync hwdge queue (DMA-only, ~7us
            # startup) carries the bf16 x stream and the deferred phase-B/H
            # weights. gpsimd software-DGE is ~single-ring slow — unused.
            wf8_sb = wp.tile([128, PT * KD * 128], F8, tag="wf8", name="wf8_sb")
            wb_sb = wp.tile([128, KD * PR], BF16, tag="wb", name="wb_sb")
            wlam_sb = wp.tile([128, PT * PR], F8, tag="wlam", name="wlam_sb")
            warm_sb = wp.tile([128, 128], BF16, tag="warm", name="warm_sb")
            wout_sb = wp.tile([128, 128], BF16, tag="wout", name="wout_sb")
            xf8_tiles, xt_tiles = [], []
            for c in range(5):
                w = KD * CHW[c]
                xf8_c = xp.tile([128, w], F8, tag=f"xf8_{c}", name=f"xf8_{c}")
                xt_c = xp.tile([128, w], BF16, tag=f"xt{c}", name=f"xt{c}")
                xf8_tiles.append(xf8_c)
                xt_tiles.append(xt_c)
            nc.scalar.dma_start(wf8_sb[:, :2048], wf8_d[:, :2048])
            nc.sync.dma_start(xf8_tiles[0][:], xf8_d[:, : KD * CHW[0]])
            nc.scalar.dma_start(wf8_sb[:, 2048:], wf8_d[:, 2048:])
            nc.gpsimd.dma_start(wb_sb[:], wb_d[:, :])
            nc.gpsimd.dma_start(wlam_sb[:], wlam_d[:, :])
            for c in range(1, 5):
                o, w = KD * COFF[c], KD * CHW[c]
                nc.scalar.dma_start(xf8_tiles[c][:], xf8_d[:, o : o + w])
            wcomb_sb = wp.tile([128, KD * P], BF16, tag="wcomb", name="wcomb_sb")
            cmat_sb = wp.tile([PR, P], BF16, tag="cmat", name="cmat_sb")
            wfp_sb = wp.tile([128, PT * D], BF16, tag="wfp", name="wfp_sb")
            for c in range(5):
                o, w = KD * COFF[c], KD * CHW[c]
                nc.sync.dma_start(xt_tiles[c][:], xt_d[:, o : o + w])
                if c == 1:
                    nc.sync.dma_start(wcomb_sb[:], wcomb_d[:, :])
            nc.sync.dma_start(cmat_sb[:], cmat_d[:, :])
            nc.sync.dma_start(
                wfp_sb[:].rearrange("p (k q) -> p k q", k=PT), wfp_v[:, :, :]
            )

            # warm the PE pipeline and the sigmoid table while DMAs land
            nc.vector.memset(warm_sb[:], 0.0)
            pw = ppa.tile([128, CH], F32, tag="pa", name="pw")
            for i in range(8):
                nc.tensor.matmul(
                    pw[:, :128], warm_sb[:], warm_sb[:],
                    start=(i == 0), stop=(i == 7),
                )
            nc.scalar.activation(wout_sb[:, 0:1], warm_sb[:, 0:1], SIG)

            # ---- persistent activations ----
            # Fg stored as fp8 DoubleRow pairs (only consumer is the lam mm)
            fgp_sb = [
                big.tile([128, 2, TE], F8, tag=f"fgp{q2}", name=f"fgp{q2}")
                for q2 in range(2)
            ]
            ttil_sb = [
                big.tile([128, TL], BF16, tag=f"ttil{m}", name=f"ttil{m}")
                for m in range(PT)
            ]
            lam_sb = big.tile([PR, TE], F32, tag="lam", name="lam")
            s1_sb = big.tile([PR, TE], F32, tag="s1", name="s1")
            sloc_sb = big.tile([PR, TL], BF16, tag="sloc", name="sloc")
            h_sb = [
                big.tile([128, hi - lo, D], BF16, tag=f"hs{i}", name=f"hs{i}")
                for i, (lo, hi) in enumerate(HPIECES)
            ]

            # ---- phase A: Fg (fp8), lam, u, chained scan over ext tokens ----
            for c in range(5):
                co, w = COFF[c], CHW[c]
                xf8_c, xt_c = xf8_tiles[c], xt_tiles[c]
                for m in range(PT):
                    pa = ppa.tile([128, CH], F32, tag="pa", name=f"pa{c}_{m}")
                    for q in range(KQ):
                        nc.tensor.matmul(
                            pa[:, :w],
                            _pairs(
                                wf8_sb[:, (m * KQ + q) * 256 : (m * KQ + q + 1) * 256],
                                128,
                            ),
                            _pairs(xf8_c[:, q * 2 * w : (q + 1) * 2 * w], w),
                            start=(q == 0),
                            stop=(q == KQ - 1),
                            perf_mode=DR,
                        )
                    nc.scalar.activation(
                        fgp_sb[m // 2][:, m % 2, co : co + w], pa[:, :w], SIG
                    )
                # u = x @ WB (bf16, 64 wide)
                pu = pps.tile([PR, CH], F32, tag="ps", name=f"pu{c}")
                for k in range(KD):
                    nc.tensor.matmul(
                        pu[:, :w],
                        wb_sb[:, k * PR : (k + 1) * PR],
                        xt_c[:, k * w : (k + 1) * w],
                        start=(k == 0),
                        stop=(k == KD - 1),
                    )
                # lam = sigmoid(Fg @ W_lam), fp8 DoubleRow over the P pairs
                pl = pps.tile([PR, CH], F32, tag="ps", name=f"pl{c}")
                for q2 in range(2):
                    nc.tensor.matmul(
                        pl[:, :w],
                        _pairs(wlam_sb[:, q2 * 128 : (q2 + 1) * 128], PR),
                        fgp_sb[q2][:, :, co : co + w],
                        start=(q2 == 0),
                        stop=(q2 == 1),
                        perf_mode=DR,
                    )
                nc.scalar.activation(lam_sb[:, co : co + w], pl[:, :w], SIG)
                # chained scan; u consumed straight from PSUM
                init = 0.0 if c == 0 else s1_sb[:, co - 1 : co]
                nc.vector.tensor_tensor_scan(
                    s1_sb[:, co : co + w], lam_sb[:, co : co + w], pu[:, :w],
                    init, op0=MUL, op1=ADD,
                )
                # main-frame bf16 copy of the scan state for the C_mat matmul
                if c == 0:
                    nc.vector.tensor_copy(sloc_sb[:, 0 : w - WU], s1_sb[:, WU:w])
                else:
                    nc.vector.tensor_copy(
                        sloc_sb[:, co - WU : co - WU + w], s1_sb[:, co : co + w]
                    )

            # ---- phase B: t_tilde = x @ W_comb + s @ C, all in PSUM ----
            for g, (c, xo, mo, w) in enumerate(BGRP):
                xt_c = xt_tiles[c]
                cw = CHW[c]
                for m in range(PT):
                    pb = ppa.tile([128, CH], F32, tag="pa", name=f"pb{g}_{m}")
                    for k in range(KD):
                        nc.tensor.matmul(
                            pb[:, :w],
                            wcomb_sb[:, k * P + m * 128 : k * P + (m + 1) * 128],
                            xt_c[:, k * cw + xo : k * cw + xo + w],
                            start=(k == 0),
                            stop=False,
                        )
                    nc.tensor.matmul(
                        pb[:, :w],
                        cmat_sb[:, m * 128 : (m + 1) * 128],
                        sloc_sb[:, mo : mo + w],
                        start=False,
                        stop=True,
                    )
                    nc.vector.tensor_copy(ttil_sb[m][:, mo : mo + w], pb[:, :w])

            # ---- phase H: h tiles, batched output DMA. All PSUM->SBUF copies
            # run on Vector so the Scalar hwdge queue stays free for its share
            # of the output piece DMAs (piece waits there would stall copies).
            for pi, (lo, hi) in enumerate(HPIECES):
                for tt in range(lo, hi):
                    ts_ = slice(tt * 128, (tt + 1) * 128)
                    ph = pph.tile([128, D], F32, tag="ph", name=f"ph{tt}")
                    for dc in range(2):
                        for k in range(PT):
                            nc.tensor.matmul(
                                ph[:, dc * CH : (dc + 1) * CH],
                                ttil_sb[k][:, ts_],
                                wfp_sb[:, k * D + dc * CH : k * D + dc * CH + CH],
                                start=(k == 0),
                                stop=(k == PT - 1),
                            )
                    nc.vector.tensor_copy(h_sb[pi][:, tt - lo, :], ph[:])
                eng = nc.sync if pi % 2 == 0 else nc.scalar
                eng.dma_start(h_v[:, lo:hi, :], h_sb[pi][:, :, :])

    nc.compile()
    return nc


def _prep_inputs(x, W_toP, W_U, W_F, W_V, W_lam, B_mat, C_mat, W_fromP):
    """Host-side sharding prep: weight folds, dtype casts, per-core x swizzle."""
    bf = ml_dtypes.bfloat16
    f8 = ml_dtypes.float8_e4m3fn

    def swz(w):
        # [K*128, q] -> partition-major [128, K*q]
        kq = w.shape[0] // 128
        return np.ascontiguousarray(
            w.reshape(kq, 128, w.shape[1]).transpose(1, 0, 2).reshape(128, -1)
        )

    W_comb = (W_toP + (W_U * W_V[None, :, :]).sum(-1)).astype(np.float32)
    WB = W_comb @ np.asarray(B_mat, np.float32)
    # W_F fp8 m-major DoubleRow pairs: wf8[p, ((m*KQ+q)*2+i)*128+j]
    wf = np.asarray(W_F, np.float32).reshape(KQ, 2, 128, PT, 128)
    wf8 = np.ascontiguousarray(
        wf.transpose(2, 3, 0, 1, 4).reshape(128, -1)
    ).astype(f8)
    wb = swz(WB).astype(bf)
    wcomb = swz(W_comb).astype(bf)
    # W_lam fp8 DoubleRow pairs: wlam8[p, (q*2+i)*64+r] = W_lam[(2q+i)*128+p, r]
    wl = np.asarray(W_lam, np.float32).reshape(2, 2, 128, PR)
    wlam = np.ascontiguousarray(
        wl.transpose(2, 0, 1, 3).reshape(128, -1)
    ).astype(f8)
    cmat = np.asarray(C_mat, np.float32).astype(bf)
    wfp = np.asarray(W_fromP, np.float32).astype(bf)
    in_maps = []
    for c in range(NCORES):
        b, half = c // 2, c % 2
        xm = np.asarray(x[b, half * TL : (half + 1) * TL, :], np.float32)
        if half == 0:
            warm = np.zeros((WU, D), np.float32)
        else:
            warm = np.asarray(x[b, TL - WU : TL, :], np.float32)
        xT = np.concatenate([warm, xm], axis=0).T       # [D, TE]
        pieces, pieces8 = [], []
        for ci in range(5):
            blk = xT[:, COFF[ci] : COFF[ci] + CHW[ci]]
            # bf16: [D, w] -> [128, KD*w] k-major partition-major
            pieces.append(
                blk.reshape(KD, 128, CHW[ci]).transpose(1, 0, 2).reshape(128, -1)
            )
            # fp8 DoubleRow: [128, (q*2+i)*w + t]
            pieces8.append(
                blk.reshape(KQ, 2, 128, CHW[ci]).transpose(2, 0, 1, 3).reshape(128, -1)
            )
        xs = np.ascontiguousarray(np.concatenate(pieces, axis=1)).astype(bf)
        xs8 = np.ascontiguousarray(np.concatenate(pieces8, axis=1)).astype(f8)
        in_maps.append(
            {
                "xt": xs,
                "xf8": xs8,
                "wf8": wf8,
                "wb": wb,
                "wcomb": wcomb,
                "wlam": wlam,
                "cmat": cmat,
                "wfp": wfp,
            }
        )
    return in_maps


def kernel(**inputs) -> np.ndarray:
    inputs = {k: np.asarray(v) for k, v in inputs.items()}
    if "nc" not in _CACHE:
        _CACHE["nc"] = build_program()
    nc = _CACHE["nc"]
    in_maps = _prep_inputs(**inputs)
    trace = bool(int(os.environ.get("CEPTA_TRACE", "0")))
    res = bass_utils.run_bass_kernel_spmd(
        nc,
        in_maps,
        core_ids=list(range(NCORES)),
        trace=trace,
        trace_cores=[0] if trace else None,
    )
    _CACHE["last_result"] = res
    out = np.empty((B, T, D), np.float32)
    for c in range(NCORES):
        b, half = c // 2, c % 2
        out[b, half * TL : (half + 1) * TL, :] = res.results[c]["h"].astype(
            np.float32
        )
    return out
